# revision 44
# baseline (speedup 1.0000x reference)
"""Causal single-head attention (B=4, T=4096, C=2048, H=128) on 8 TRN2 cores.

Sharding: data-parallel over batch (2 cores per batch element). Within a
batch, core half h owns query tiles qt with qt mod 4 in {2h, 2h+1} — both
cores get an identical multiset of causal key-block counts, so one SPMD
program is balanced. Each core also projects k/v/q only for its own 2048
columns; (k^T | v^T) halves are exchanged with a pair-wise AllGather per
TWO 512-column groups (4 collectives total), halving both the x DMA
traffic and the k/v projection FLOPs.

Per-core device program (fp16 operands, f32 PSUM accumulation), pipelined
per column group g: project k^T / v^T / q^T of my 256 columns from slab
g, AllGather (k^T | v^T) of groups (2i, 2i+1) with the pair partner, and
interleave one attention q-group (4 query tiles, 512 q columns) per two
projection groups, in the transposed S^T layout per 256-key chunk pair:
  S^T pair [s=2x128, q=512] (PE) -> exp (ACT) -> x 0/1 causal mask (DVE,
  diagonal chunks only) -> row-sums via ones-matmul + out^T AV
  accumulation (PE) -> out^T * (1/sums) (DVE approx-recip + mul) ->
  +bv, cast (ACT, per-partition bias in the out^T layout) -> DMA out^T.
The output is returned transposed ([H, TQ] per core); the host undoes it.
"""

import ml_dtypes
import numpy as np

import concourse.bacc as bacc
import concourse.mybir as mybir
import concourse.tile as tile
from concourse.bass_utils import run_bass_kernel_spmd

B, T, C, H = 4, 4096, 2048, 128
P = 128          # partitions / head dim / q tile
KB = 512         # free-dim tile (one f32 PSUM bank)
HB = 256         # per-core half of a column group
NQT = 16         # query tiles per core
TQ = NQT * P     # query rows per core
NCC = C // P     # contraction chunks (16)
NG = T // KB     # 512-wide column groups (8)
NM = 4           # attention q-groups per core (4 tiles each)
NMSK = 8 * NM    # 8 masked 128-key chunks per q-group

F16 = np.float16
F8 = ml_dtypes.float8_e4m3fn
_NC_CACHE = {}
REPLICA_GROUPS = [[0, 1], [2, 3], [4, 5], [6, 7]]


def _qtiles_for(half):
    # global query-tile ids, j-th tile of this core; kb counts [1,1,2,2,...,8,8]
    return [4 * (j // 2) + 2 * half + (j % 2) for j in range(NQT)]


def build_nc():
    dt = mybir.dt
    nc = bacc.Bacc("TRN2", target_bir_lowering=False, debug=False, num_devices=8)

    xP = nc.dram_tensor("xP", [NG, P, NCC, HB], dt.float16, kind="ExternalInput").ap()
    wk = nc.dram_tensor("wk", [P, NCC, H], dt.float16, kind="ExternalInput").ap()
    wq = nc.dram_tensor("wq", [P, NCC, H], dt.float16, kind="ExternalInput").ap()
    wv = nc.dram_tensor("wv", [P, NCC, H], dt.float16, kind="ExternalInput").ap()
    bk = nc.dram_tensor("bk", [P, 1], dt.float32, kind="ExternalInput").ap()
    bq = nc.dram_tensor("bq", [P, 1], dt.float32, kind="ExternalInput").ap()
    bv = nc.dram_tensor("bv", [P, 1], dt.float32, kind="ExternalInput").ap()
    consts = nc.dram_tensor("consts", [P, 2, P], dt.float16, kind="ExternalInput").ap()
    masks = nc.dram_tensor("masks", [P, NMSK, KB], dt.float16, kind="ExternalInput").ap()
    outT = nc.dram_tensor("outT", [P, NM, KB], dt.float32, kind="ExternalOutput").ap()

    Exp = mybir.ActivationFunctionType.Exp
    Ident = mybir.ActivationFunctionType.Identity

    with tile.TileContext(nc) as tc:
        with (
            tc.tile_pool(name="wpool", bufs=1) as wpool,
            tc.tile_pool(name="cpool", bufs=1) as cpool,
            tc.tile_pool(name="persist", bufs=1) as persist,
            tc.tile_pool(name="mpool", bufs=1) as mpool,
            tc.tile_pool(name="xpool", bufs=5) as xpool,
            tc.tile_pool(name="accpool", bufs=2) as accpool,
            tc.tile_pool(name="vtpool", bufs=2) as vtpool,
            tc.tile_pool(name="kvpool", bufs=2) as kvpool,
            tc.tile_pool(name="dram", bufs=4, space="DRAM") as dram,
            tc.tile_pool(name="ppool", bufs=2, space="PSUM") as ppool,
            tc.tile_pool(name="spool", bufs=2, space="PSUM") as spool,
            tc.tile_pool(name="sumpool", bufs=1, space="PSUM") as sumpool,
            tc.tile_pool(name="otpool", bufs=1, space="PSUM") as otpool,
            tc.tile_pool(name="weipool", bufs=5) as weipool,
            tc.tile_pool(name="statpool", bufs=2) as statpool,
            tc.tile_pool(name="osbpool", bufs=2) as osbpool,
            tc.tile_pool(name="opool", bufs=2) as opool,
        ):
            # ---- constants (tiny ones first so slab 0 lands early) ----
            bk_t = cpool.tile([P, 1], dt.float32, tag="bk")
            bq_t = cpool.tile([P, 1], dt.float32, tag="bq")
            bv_t = cpool.tile([P, 1], dt.float32, tag="bv")
            idon = cpool.tile([P, 2, P], dt.float16, tag="idon")
            nc.sync.dma_start(bk_t[:], bk)
            nc.sync.dma_start(bq_t[:], bq)
            nc.sync.dma_start(bv_t[:], bv)
            nc.sync.dma_start(idon[:], consts)
            wk_t = wpool.tile([P, NCC, H], dt.float16, tag="wk")
            wq_t = wpool.tile([P, NCC, H], dt.float16, tag="wq")
            wv_t = wpool.tile([P, NCC, H], dt.float16, tag="wv")
            nc.sync.dma_start(wk_t[:], wk)
            masks_t = mpool.tile([P, NMSK, KB], dt.float16, tag="masks")
            nc.scalar.dma_start(masks_t[:], masks)

            kT = persist.tile([P, T], dt.float16, tag="kT")
            qT = persist.tile([P, TQ], dt.float16, tag="qT")
            vS = persist.tile([P, T // P, H], dt.float16, tag="vS")

            DR = mybir.MatmulPerfMode.DoubleRow
            fetched = {}

            def fetch(g):
                # DMA slab g and cast it to fp8 (issued ahead of use so the
                # cast sits early in the DVE queue)
                if g >= NG or g in fetched:
                    return
                xs = xpool.tile([P, NCC, HB], dt.float16, tag="xs", name=f"xs{g}")
                nc.sync.dma_start(xs[:], xP[g])
                fetched[g] = xs

            def project(g, kv2):
                xs = fetched.pop(g)
                fetch(g + 3)
                if g == 3:
                    fetch(7)
                gg = g % 2
                # k^T of my half
                pk = ppool.tile([P, HB], dt.float32, tag="proj")
                for cc in range(NCC):
                    nc.tensor.matmul(
                        pk[:], lhsT=wk_t[:, cc, :], rhs=xs[:, cc, :],
                        start=(cc == 0), stop=(cc == NCC - 1),
                    )
                nc.scalar.activation(kv2[:, gg, 0, :], pk[:], Ident, bias=bk_t[:])
                # q^T for my two tiles
                pq = ppool.tile([P, HB], dt.float32, tag="proj")
                for cc in range(NCC):
                    nc.tensor.matmul(
                        pq[:], lhsT=wq_t[:, cc, :], rhs=xs[:, cc, :],
                        start=(cc == 0), stop=(cc == NCC - 1),
                    )
                nc.scalar.activation(
                    qT[:, HB * g : HB * (g + 1)], pq[:], Ident, bias=bq_t[:],
                )
                # v^T of my half (fp16)
                pv = ppool.tile([P, HB], dt.float32, tag="proj")
                for cc in range(NCC):
                    nc.tensor.matmul(
                        pv[:], lhsT=wv_t[:, cc, :], rhs=xs[:, cc, :],
                        start=(cc == 0), stop=(cc == NCC - 1),
                    )
                vt = vtpool.tile([P, HB], dt.float16, tag="vt")
                nc.scalar.copy(vt[:], pv[:])
                for s4 in range(2):
                    tp = ppool.tile([P, P], dt.float16, tag="proj")
                    nc.tensor.transpose(
                        tp[:], vt[:, P * s4 : P * (s4 + 1)], idon[:, 0, :]
                    )
                    nc.vector.tensor_copy(
                        kv2[:, gg, 1, P * s4 : P * (s4 + 1)], tp[:]
                    )

            def exchange(i):
                # AllGather (k^T | v^T) of groups 2i, 2i+1 with the pair partner.
                # cin staging on the scalar ring: the trigger lands right after
                # the k/q activations that produce kv, so it neither waits long
                # nor blocks the x-slab stream on the sync ring.
                cin = dram.tile([P, 2, 2, HB], dt.float16, tag="cin")
                cout = dram.tile([2, P, 2, 2, HB], dt.float16, tag="cout")
                nc.scalar.dma_start(cin[:], kvx[i % 2][:])
                nc.gpsimd.collective_compute(
                    "AllGather",
                    mybir.AluOpType.bypass,
                    replica_groups=REPLICA_GROUPS,
                    ins=[cin.opt()],
                    outs=[cout.opt()],
                )
                return cout

            def unpack(i, cout):
                # cout[r, :, gg] = (k^T | v^T) of group 2i+gg from core-half r
                # sync ring: the x-slab stream is fully prefetched before the
                # first unpack, so these never block it — and HWDGE is much
                # faster than the gpsimd SWDGE (whose queue must stay free for
                # the back-to-back collectives)
                for gg in range(2):
                    g = 2 * i + gg
                    nc.sync.dma_start(
                        kT[:, KB * g : KB * (g + 1)].rearrange(
                            "p (r h) -> p r h", r=2
                        ),
                        cout[:, :, gg, 0, :].rearrange("r p h -> p r h"),
                    )
                    nc.sync.dma_start(
                        vS[:, 4 * g : 4 * (g + 1), :].rearrange(
                            "p (r s) h -> p r s h", r=2
                        ),
                        cout[:, :, gg, 1, :].rearrange(
                            "r p (s h) -> p r s h", s=2
                        ),
                    )

            def attention(m):
                nch = 8 * m + 8     # 128-wide key chunks for this group
                npr = nch // 2
                sums = sumpool.tile([P, KB], dt.float32, tag="sums")
                otp = otpool.tile([P, KB], dt.float32, tag="otp")
                acc = accpool.tile([P, 2, KB], dt.float16, tag="acc")
                qg = qT[:, KB * m : KB * (m + 1)]
                wei_tiles = []

                def av(p):
                    w = wei_tiles[p]
                    for h2 in range(2):
                        c = 2 * p + h2
                        nc.tensor.matmul(
                            otp[:], lhsT=vS[:, c, :], rhs=w[:, h2, :],
                            start=(c == 0), stop=(c == nch - 1),
                        )

                for p in range(npr):
                    st = spool.tile([P, 2, KB], dt.float32, tag="st")
                    for h2 in range(2):
                        nc.tensor.matmul(
                            st[:, h2, :],
                            lhsT=kT[:, P * (2 * p + h2) : P * (2 * p + h2 + 1)],
                            rhs=qg, start=True, stop=True,
                        )
                    wei = weipool.tile([P, 2, KB], dt.float16, tag="wei")
                    nc.scalar.activation(wei[:], st[:], Exp)
                    if p >= npr - 4:
                        k8 = 2 * (p - (npr - 4))
                        nc.vector.tensor_mul(
                            wei[:], wei[:],
                            masks_t[:, 8 * m + k8 : 8 * m + k8 + 2, :],
                        )
                    # row-sum side: accumulate exp tiles on the DVE instead of
                    # spending a PE matmul per chunk; one partition-reduce
                    # matmul pair at the end produces the replicated sums
                    if p == 0:
                        nc.vector.tensor_copy(acc[:], wei[:])
                    else:
                        nc.vector.tensor_add(acc[:], acc[:], wei[:])
                    wei_tiles.append(wei)
                    if p > 0:
                        av(p - 1)
                av(npr - 1)
                for h2 in range(2):
                    nc.tensor.matmul(
                        sums[:], lhsT=idon[:, 1, :], rhs=acc[:, h2, :],
                        start=(h2 == 0), stop=(h2 == 1),
                    )
                rec = statpool.tile([P, KB], dt.float32, tag="rec")
                nc.vector.reciprocal_approx_fast(rec[:], sums[:])
                osb = osbpool.tile([P, KB], dt.float16, tag="osb")
                nc.vector.tensor_mul(osb[:], otp[:], rec[:])
                oT = opool.tile([P, KB], dt.float32, tag="oT")
                nc.scalar.activation(oT[:], osb[:], Ident, bias=bv_t[:])
                # scalar ring: fires right after the producing activation,
                # keeps the sync ring free for the x-slab stream
                nc.scalar.dma_start(outT[:, m, :], oT[:])

            # warm-up collective: absorbs CC-core boot + barrier skew while
            # the x/weight DMAs stream in
            cin_w = dram.tile([P, 2], dt.float16, tag="cinw")
            cout_w = dram.tile([2, P, 2], dt.float16, tag="coutw")
            nc.gpsimd.dma_start(cin_w[:], idon[:, 0, 0:2])
            nc.gpsimd.collective_compute(
                "AllGather",
                mybir.AluOpType.bypass,
                replica_groups=REPLICA_GROUPS,
                ins=[cin_w.opt()],
                outs=[cout_w.opt()],
            )

            # pipeline: projections feed attention groups as kT/vS fill in
            kvx = [
                kvpool.tile([P, 2, 2, HB], dt.float16, tag="kvx", name="kvx0"),
                kvpool.tile([P, 2, 2, HB], dt.float16, tag="kvx", name="kvx1"),
            ]
            fetch(0)
            nc.sync.dma_start(wq_t[:], wq)
            nc.sync.dma_start(wv_t[:], wv)
            fetch(1)
            fetch(2)
            project(0, kvx[0])
            project(1, kvx[0])
            co0 = exchange(0)
            project(2, kvx[1])
            project(3, kvx[1])
            co1 = exchange(1)
            unpack(0, co0)
            attention(0)
            project(4, kvx[0])
            project(5, kvx[0])
            co2 = exchange(2)
            unpack(1, co1)
            attention(1)
            project(6, kvx[1])
            project(7, kvx[1])
            co3 = exchange(3)
            unpack(2, co2)
            unpack(3, co3)
            attention(2)
            attention(3)

    nc.compile()
    return nc


def _host_prep(x, Wk, bk, Wq, bq, Wv, bv):
    scale = float(C) ** -0.5

    def tile_w(w):
        return np.ascontiguousarray(
            w.reshape(NCC, P, H).transpose(1, 0, 2)
        )

    wk16 = tile_w(np.asarray(Wk, np.float32).astype(F16))
    wq16 = tile_w((np.asarray(Wq, np.float32) * scale).astype(F16))
    wv16 = tile_w(np.asarray(Wv, np.float32).astype(F16))
    bk_c = np.asarray(bk, np.float32).reshape(P, 1)
    bq_c = (np.asarray(bq, np.float32) * scale).reshape(P, 1)
    bv_c = np.asarray(bv, np.float32).reshape(P, 1)
    consts = np.ascontiguousarray(
        np.stack([np.eye(P, dtype=F16), np.ones((P, P), F16)]).transpose(1, 0, 2)
    )

    # masks per half: key order is natural global t; 0/1 multiplicative
    masks_by_half = []
    for half in (0, 1):
        qts = _qtiles_for(half)
        m_arr = np.zeros((P, NMSK, KB), F16)
        for m in range(NM):
            nch = 8 * m + 8
            qrow = np.empty(KB, np.int64)
            for r in range(4):
                qrow[P * r : P * (r + 1)] = qts[4 * m + r] * P + np.arange(P)
            for k in range(8):
                c = (nch - 8) + k
                keys = P * c + np.arange(P)
                m_arr[:, 8 * m + k, :] = (
                    keys[:, None] <= qrow[None, :]
                ).astype(F16)
        masks_by_half.append(m_arr)

    in_maps = []
    for core in range(8):
        b_idx, half = core // 2, core % 2
        xTb = np.ascontiguousarray(np.asarray(x[b_idx], np.float32).T)
        xPc = np.empty((NG, P, NCC, HB), F16)
        for g in range(NG):
            grp = xTb[:, KB * g + HB * half : KB * g + HB * (half + 1)]
            xPc[g] = grp.reshape(NCC, P, HB).transpose(1, 0, 2).astype(F16)
        in_maps.append({
            "xP": xPc,
            "wk": wk16, "wq": wq16, "wv": wv16,
            "bk": bk_c, "bq": bq_c, "bv": bv_c,
            "consts": consts, "masks": masks_by_half[half],
        })
    return in_maps


def kernel(x, Wk, bk, Wq, bq, Wv, bv):
    if "nc" not in _NC_CACHE:
        _NC_CACHE["nc"] = build_nc()
    nc = _NC_CACHE["nc"]
    in_maps = _host_prep(x, Wk, bk, Wq, bq, Wv, bv)
    res = run_bass_kernel_spmd(nc, in_maps, list(range(8))).results
    out = np.empty((B, T, H), np.float32)
    for core in range(8):
        b_idx, half = core // 2, core % 2
        oT = res[core]["outT"]  # [P(H), NM, KB]
        qts = _qtiles_for(half)
        for j, qt in enumerate(qts):
            m, r = j // 4, j % 4
            out[b_idx, qt * P : (qt + 1) * P, :] = oT[:, m, r * P : (r + 1) * P].T
    return out
